# revision 1
# baseline (speedup 1.0000x reference)
"""Trainium2 Bass kernel for a causal self-attention block (GQA + per-head
RMS-norm + RoPE + learned q-gain), sharded over 8 NeuronCores.

Sharding: data-parallel over batch (B=2) as the outer axis x tensor-parallel
over head groups (4 groups of 4 query heads, each owning one KV head).
core = b*4 + g. Each core computes the full attention for its 4 heads and a
*partial* output projection (its 256 in-dims of Wproj); the host sums the 4
partials per batch element and transposes back.

Inside a core everything is computed in a transposed ("T") layout for the
attention matmuls: scores are built as S^T[k, q] = K @ Q^T so the PV matmul
can contract over keys on the partition axis; a row of ones appended to V
yields the softmax denominator for free.
"""

import math

import numpy as np

import concourse.bacc as bacc
import concourse.bass as bass
import concourse.tile as tile
from concourse import mybir
from concourse.bass import ts
from concourse.bass_utils import run_bass_kernel_spmd
from concourse.masks import make_identity

# Problem dims (hardcoded per contract).
B, S, D, H, KV, HD = 2, 2048, 1024, 16, 4, 64
NH = H // KV          # 4 query heads per core (one KV group)
GD = NH * HD          # 256 out-dims of Wq per group
P = 128               # partitions
NST = S // P          # 16 sequence tiles
JW = 512              # query-block width for attention
NJ = S // JW          # 4 query blocks
NC = 8                # cores
ROPE_BASE = 10000.0
RMS_EPS = 1.1920929e-07
F32 = mybir.dt.float32
F32R = mybir.dt.float32r
AXX = mybir.AxisListType.X
ACT = mybir.ActivationFunctionType


def _build_program(reps=1):
    # Bacc (vs raw Bass) runs the TRN2 lowering passes: matmul waits moved to
    # ldweights, sync-wait splitting, act-table/library load insertion.
    # reps>1 replicates the whole body for slope-based benchmarking.
    nc = bacc.Bacc("TRN2", target_bir_lowering=False, debug=False)

    # f32r tensors hold ordinary f32 bits; the declaration lets the PE run
    # its full-rate fp32 path (4x faster than strict fp32 matmul).
    xT = nc.dram_tensor("xT", [D, S], F32R, kind="ExternalInput").ap()
    wqkv = nc.dram_tensor("wqkv", [D, GD + 2 * HD], F32R, kind="ExternalInput").ap()
    wp2 = nc.dram_tensor("wp2", [P, 2 * D], F32R, kind="ExternalInput").ap()
    cosn = nc.dram_tensor("cosn", [P, NST * HD], F32, kind="ExternalInput").ap()
    sinn = nc.dram_tensor("sinn", [P, NST * 32], F32, kind="ExternalInput").ap()
    masks = nc.dram_tensor("masks", [P, 4 * JW], F32, kind="ExternalInput").ap()
    qg8 = nc.dram_tensor("qg8", [1, NH], F32, kind="ExternalInput").ap()
    ypt = nc.dram_tensor("ypt", [D, S], F32, kind="ExternalOutput").ap()

    with tile.TileContext(nc) as tc:
        for _ in range(reps):
            _body(tc, xT, wqkv, wp2, cosn, sinn, masks, qg8, ypt)
    nc.compile()
    return nc


def _body(tc, xT, wqkv, wp2, cosn, sinn, masks, qg8, ypt):
    nc = tc.nc
    NQKV = GD + 2 * HD  # 384

    with tc.tile_pool(name="consts", bufs=1) as consts:
        # Persistent SBUF state.
        wp_sb = consts.tile([P, 2, D], F32R, name="wp_sb")
        cos_sb = consts.tile([P, NST, HD], F32, name="cos_sb")
        sin_sb = consts.tile([P, NST, 32], F32, name="sin_sb")
        mask_sb = consts.tile([P, 4, JW], F32, name="mask_sb")
        qg8_sb = consts.tile([P, NH], F32, name="qg8_sb")
        ident = consts.tile([P, P], F32, name="ident")
        # qT/kT zero-padded to 128 partitions so attention matmuls run K=128
        # (no PE tiling-mode switches); rows 64-127 stay zero.
        qT_sb = consts.tile([P, NH, S], F32R, name="qT_sb")
        kT_sb = consts.tile([P, S], F32R, name="kT_sb")
        # V with a ones-column (65th) so PV accumulates softmax denominators.
        v_sb = consts.tile([P, NST, HD + 1], F32R, name="v_sb")
        # Normalized y^T, head pairs stacked on partitions for the out-proj.
        y_sb = consts.tile([P, 2, S], F32R, name="y_sb")
        # Selector matrix (row 64 all-ones) broadcasts the softmax denominator
        # over partitions via a plain K=128 matmul (no PE mode switch).
        sel64 = consts.tile([P, P], F32R, name="sel64")
        # Rotating staging rows for the reciprocal denominators: rows 0-63 and
        # 65-127 stay zero forever; row 64 is rewritten per use.
        bc0 = consts.tile([P, JW], F32R, name="bc0")
        bc1 = consts.tile([P, JW], F32R, name="bc1")

        nc.sync.dma_start(out=wp_sb, in_=wp2.rearrange("p (c m) -> p c m", c=2))
        nc.sync.dma_start(out=cos_sb, in_=cosn.rearrange("p (t f) -> p t f", f=HD))
        nc.sync.dma_start(out=sin_sb, in_=sinn.rearrange("p (t f) -> p t f", f=32))
        nc.sync.dma_start(out=mask_sb, in_=masks.rearrange("p (m c) -> p m c", c=JW))
        nc.gpsimd.dma_start(out=qg8_sb, in_=qg8.to_broadcast([P, NH]))
        make_identity(nc, ident)
        # f32r tiles can't be memset directly (ISA); fill via f32 -> f32r
        # broadcast copies, which are legal rounding producers.
        z1 = consts.tile([P, 1], F32, name="z1")
        o1 = consts.tile([P, 1], F32, name="o1")
        nc.vector.memset(z1, 0.0)
        nc.vector.memset(o1, 1.0)
        nc.vector.tensor_copy(
            v_sb[:, :, HD : HD + 1], o1[:, None, :].broadcast_to([P, NST, 1])
        )
        nc.vector.tensor_copy(
            qT_sb[HD:P, :, :], z1[HD:P, :][:, None, :].broadcast_to([HD, NH, S])
        )
        nc.vector.tensor_copy(kT_sb[HD:P, :], z1[HD:P, :].broadcast_to([HD, S]))
        nc.vector.tensor_copy(sel64, z1.broadcast_to([P, P]))
        nc.vector.tensor_copy(
            sel64[HD : HD + 1, :], o1[HD : HD + 1, :].broadcast_to([1, P])
        )
        nc.vector.tensor_copy(bc0, z1.broadcast_to([P, JW]))
        nc.vector.tensor_copy(bc1, z1.broadcast_to([P, JW]))

        # ---------------- Phase 1: QKV proj + RMS + RoPE + transposes -----
        with tc.tile_pool(name="ph1c", bufs=1) as ph1c:
            xT_sb = ph1c.tile([P, 8, S], F32R, name="xT_sb")
            w_sb = ph1c.tile([P, 8, NQKV], F32R, name="w_sb")
            xTr = xT.rearrange("(c p) s -> p c s", p=P)
            for c in range(8):
                nc.sync.dma_start(out=xT_sb[:, c, :], in_=xTr[:, c, :])
            nc.sync.dma_start(out=w_sb, in_=wqkv.rearrange("(c p) n -> p c n", p=P))

            with (
                tc.tile_pool(name="p1w", bufs=3) as work,
                tc.tile_pool(name="p1ps", bufs=3, space="PSUM") as psP,
                tc.tile_pool(name="p1pt", bufs=4, space="PSUM") as psT,
            ):
                for i in range(NST):
                    qkv_ps = psP.tile([P, NQKV], F32, name=f"qkv_ps{i}", tag="qkv")
                    for c in range(8):
                        nc.tensor.matmul(
                            qkv_ps,
                            lhsT=xT_sb[:, c, ts(i, P)],
                            rhs=w_sb[:, c, :],
                            start=(c == 0),
                            stop=(c == 7),
                        )
                    # V tile straight out of PSUM.
                    nc.scalar.copy(v_sb[:, i, 0:HD], qkv_ps[:, GD + HD : NQKV])

                    # Merged RMS stats for 4 q heads + k (5 slabs of 64).
                    sq5 = work.tile([P, 5 * HD], F32, name=f"sq5_{i}", tag="sq5")
                    nc.scalar.square(sq5, qkv_ps[:, 0 : 5 * HD])
                    ss5 = work.tile([P, 5], F32, name=f"ss5_{i}", tag="ss5")
                    nc.vector.reduce_sum(
                        ss5, sq5.rearrange("p (h d) -> p h d", d=HD), axis=AXX
                    )
                    m5 = work.tile([P, 5], F32, name=f"m5_{i}", tag="m5")
                    nc.vector.tensor_scalar(
                        out=m5, in0=ss5, scalar1=1.0 / HD, scalar2=RMS_EPS,
                        op0=mybir.AluOpType.mult, op1=mybir.AluOpType.add,
                    )
                    s5 = work.tile([P, 5], F32, name=f"s5_{i}", tag="s5")
                    nc.scalar.activation(s5, m5, ACT.Sqrt)
                    r5 = work.tile([P, 5], F32, name=f"r5_{i}", tag="r5")
                    nc.vector.reciprocal(r5, s5)
                    # One Newton step on rsqrt corrects sqrt-table + recip err.
                    t5 = work.tile([P, 5], F32, name=f"t5_{i}", tag="t5")
                    nc.vector.tensor_mul(t5, m5, r5)
                    nc.vector.tensor_mul(t5, t5, r5)
                    nc.vector.tensor_scalar(
                        out=t5, in0=t5, scalar1=-0.5, scalar2=1.5,
                        op0=mybir.AluOpType.mult, op1=mybir.AluOpType.add,
                    )
                    nc.vector.tensor_mul(r5, r5, t5)
                    # Fold gain/8 into the q scales (k slab untouched).
                    nc.vector.tensor_mul(r5[:, 0:NH], r5[:, 0:NH], qg8_sb)

                    # Scale + RoPE, q and k batched (cos table duplicated to
                    # 64 wide; rot = qks*cosd then +/- the swapped-half * sin).
                    q5 = qkv_ps[:, 0 : 5 * HD].rearrange("p (h d) -> p h d", d=HD)
                    qks = work.tile([P, 5, HD], F32, name=f"qks_{i}", tag="qks")
                    nc.vector.tensor_mul(
                        qks, q5, r5[:, :, None].broadcast_to([P, 5, HD])
                    )
                    rot = work.tile([P, 5, HD], F32, name=f"rot_{i}", tag="rot")
                    cb = cos_sb[:, i, :][:, None, :].broadcast_to([P, 5, HD])
                    sb_ = sin_sb[:, i, :][:, None, :].broadcast_to([P, 5, 32])
                    nc.vector.tensor_mul(rot, qks, cb)
                    m2a = work.tile([P, 5, 32], F32, name=f"m2a_{i}", tag="m2a")
                    nc.vector.tensor_mul(m2a, qks[:, :, 32:HD], sb_)
                    m2b = work.tile([P, 5, 32], F32, name=f"m2b_{i}", tag="m2b")
                    nc.vector.tensor_mul(m2b, qks[:, :, 0:32], sb_)
                    nc.vector.tensor_add(rot[:, :, 0:32], rot[:, :, 0:32], m2a)
                    nc.vector.tensor_sub(rot[:, :, 32:HD], rot[:, :, 32:HD], m2b)

                    # Transpose each slab to [d, s] layout.
                    for slab in range(5):
                        trq = psT.tile([HD, P], F32, name=f"tr{i}_{slab}", tag="tr")
                        nc.tensor.transpose(trq, rot[:, slab, :], ident)
                        if slab < NH:
                            nc.vector.tensor_copy(qT_sb[0:HD, slab, ts(i, P)], trq)
                        else:
                            nc.vector.tensor_copy(kT_sb[0:HD, ts(i, P)], trq)

        # ---------------- Phase 2: attention --------------------------------
        with (
            tc.tile_pool(name="p2w", bufs=3) as workp,
            tc.tile_pool(name="p2s", bufs=2, space="PSUM") as psS,
            tc.tile_pool(name="p2y", bufs=2, space="PSUM") as psY,
            tc.tile_pool(name="p2b", bufs=2, space="PSUM") as psB,
        ):
            for h in range(NH):
                for j in range(NJ):
                    nt = 4 * (j + 1)  # valid k-tiles for this q block
                    y_ps = psY.tile([HD + 1, JW], F32, name=f"y_ps{h}_{j}", tag="y")
                    qh = qT_sb[:, h, ts(j, JW)]
                    for cc in range(nt // 2):
                        st = psS.tile([P, 2 * JW], F32, name=f"st{h}_{j}_{cc}", tag="st")
                        for u in range(2):
                            t = 2 * cc + u
                            nc.tensor.matmul(
                                st[:, ts(u, JW)],
                                lhsT=kT_sb[:, ts(t, P)],
                                rhs=qh,
                                start=True,
                                stop=True,
                            )
                        p_sb = workp.tile([P, 2 * JW], F32R, name=f"p{h}_{j}_{cc}", tag="p")
                        nc.scalar.activation(p_sb, st, ACT.Exp)
                        for u in range(2):
                            m = 2 * cc + u - 4 * j
                            if m >= 0:  # diagonal tile: zero the future keys
                                nc.vector.tensor_mul(
                                    p_sb[:, ts(u, JW)], p_sb[:, ts(u, JW)],
                                    mask_sb[:, m, :],
                                )
                        for u in range(2):
                            t = 2 * cc + u
                            nc.tensor.matmul(
                                y_ps,
                                lhsT=v_sb[:, t, :],
                                rhs=p_sb[:, ts(u, JW)],
                                start=(t == 0),
                                stop=(t == nt - 1),
                            )
                    # Softmax normalization: row HD of y_ps is the denom.
                    bc = (bc0, bc1)[(h * NJ + j) % 2]
                    lrow = workp.tile([HD + 1, JW], F32, name=f"lr{h}_{j}", tag="lrow")
                    nc.vector.reciprocal(lrow[HD : HD + 1, :], y_ps[HD : HD + 1, :])
                    nc.vector.tensor_copy(bc[HD : HD + 1, :], lrow[HD : HD + 1, :])
                    bcp = psB.tile([P, JW], F32, name=f"bcp{h}_{j}", tag="bcp")
                    nc.tensor.matmul(
                        bcp, lhsT=sel64, rhs=bc, start=True, stop=True
                    )
                    bcs = workp.tile([HD, JW], F32, name=f"bcs{h}_{j}", tag="bcs")
                    nc.vector.tensor_copy(bcs, bcp[0:HD, :])
                    if h % 2 == 0:
                        nc.vector.tensor_mul(
                            y_sb[0:HD, h // 2, ts(j, JW)], y_ps[0:HD, :], bcs
                        )
                    else:
                        ytmp = workp.tile([HD, JW], F32R, name=f"yt{h}_{j}", tag="ytmp")
                        nc.vector.tensor_mul(ytmp, y_ps[0:HD, :], bcs)
                        nc.sync.dma_start(
                            out=y_sb[HD:P, h // 2, ts(j, JW)], in_=ytmp
                        )

        # ---------------- Phase 3: output projection (partial) --------------
        with (
            tc.tile_pool(name="p3w", bufs=4) as worko,
            tc.tile_pool(name="p3ps", bufs=4, space="PSUM") as psO,
        ):
            for m in range(D // P):
                for j in range(NJ):
                    op_ps = psO.tile([P, JW], F32, name=f"op{m}_{j}", tag="op")
                    for c in range(2):
                        nc.tensor.matmul(
                            op_ps,
                            lhsT=wp_sb[:, c, ts(m, P)],
                            rhs=y_sb[:, c, ts(j, JW)],
                            start=(c == 0),
                            stop=(c == 1),
                        )
                    o_sb = worko.tile([P, JW], F32, name=f"o{m}_{j}", tag="o")
                    eng = nc.vector if (m + j) % 2 == 0 else nc.scalar
                    if eng is nc.vector:
                        nc.vector.tensor_copy(o_sb, op_ps)
                    else:
                        nc.scalar.copy(o_sb, op_ps)
                    nc.sync.dma_start(out=ypt[ts(m, P), ts(j, JW)], in_=o_sb)


_PROG = None


def _get_program():
    global _PROG
    if _PROG is None:
        _PROG = _build_program()
    return _PROG


def _host_tables():
    inv_freq = (1.0 / (ROPE_BASE ** (np.arange(0, HD, 2, dtype=np.float32) / HD))).astype(
        np.float32
    )
    t = np.arange(S, dtype=np.float32)
    freqs = t[:, None] * inv_freq[None, :]  # [S, 32]
    cosf = np.cos(freqs).astype(np.float32)
    sinf = np.sin(freqs).astype(np.float32)
    # natural per-s-tile layout: [p, tile, freq]
    cosd = np.concatenate([cosf, cosf], axis=1)  # [S, 64]
    cosn = np.ascontiguousarray(
        cosd.reshape(NST, P, HD).transpose(1, 0, 2).reshape(P, NST * HD)
    )
    sinn = np.ascontiguousarray(
        sinf.reshape(NST, P, 32).transpose(1, 0, 2).reshape(P, NST * 32)
    )
    p_idx = np.arange(P)[:, None]
    c_idx = np.arange(JW)[None, :]
    mlist = [(c_idx >= m * P + p_idx).astype(np.float32) for m in range(4)]
    masks = np.ascontiguousarray(np.concatenate(mlist, axis=1))  # [128, 2048]
    return cosn, sinn, masks


def _in_maps(x, Wq, Wk, Wv, Wproj, q_gain):
    cosn, sinn, masks = _host_tables()
    maps = []
    for core in range(NC):
        b, g = divmod(core, KV)
        xT = np.ascontiguousarray(x[b].T)  # [D, S]
        wqkv = np.ascontiguousarray(
            np.concatenate(
                [
                    Wq[g * GD : (g + 1) * GD].T,
                    Wk[g * HD : (g + 1) * HD].T,
                    Wv[g * HD : (g + 1) * HD].T,
                ],
                axis=1,
            )
        )  # [D, 384]
        wsl = Wproj[:, g * GD : (g + 1) * GD].T.reshape(NH, HD, D)  # [head, d, m]
        wp2 = np.ascontiguousarray(
            np.stack(
                [
                    np.concatenate([wsl[0], wsl[1]], axis=0),
                    np.concatenate([wsl[2], wsl[3]], axis=0),
                ],
                axis=1,
            ).reshape(P, 2 * D)
        )
        qg8 = np.ascontiguousarray(
            (q_gain[g * NH : (g + 1) * NH] / 8.0).astype(np.float32).reshape(1, NH)
        )
        maps.append(
            {
                "xT": xT,
                "wqkv": wqkv,
                "wp2": wp2,
                "cosn": cosn,
                "sinn": sinn,
                "masks": masks,
                "qg8": qg8,
            }
        )
    return maps


def kernel(x, Wq, Wk, Wv, Wproj, q_gain, _collect=None):
    x = np.asarray(x, dtype=np.float32)
    Wq = np.asarray(Wq, dtype=np.float32)
    Wk = np.asarray(Wk, dtype=np.float32)
    Wv = np.asarray(Wv, dtype=np.float32)
    Wproj = np.asarray(Wproj, dtype=np.float32)
    q_gain = np.asarray(q_gain, dtype=np.float32)

    nc = _get_program()
    maps = _in_maps(x, Wq, Wk, Wv, Wproj, q_gain)
    res = run_bass_kernel_spmd(nc, maps, core_ids=list(range(NC)))
    if _collect is not None:
        _collect.append(res)

    out = np.zeros((B, S, D), dtype=np.float64)
    for core in range(NC):
        b, _ = divmod(core, KV)
        out[b] += res.results[core]["ypt"].T.astype(np.float64)
    return out.astype(np.float32)



# revision 3
# speedup vs baseline: 1.2452x; 1.2452x over previous
"""Trainium2 Bass kernel for a causal self-attention block (GQA + per-head
RMS-norm + RoPE + learned q-gain), sharded over 8 NeuronCores.

Sharding: data-parallel over batch (B=2) x tensor-parallel over KV groups
(4 groups of 4 query heads). core = b*4 + g. Each core computes full
attention for its 4 heads and a partial output projection (its 256 in-dims
of Wproj); the host sums the 4 partials per batch element.

v2 design (vs the fp32r v1):
- bf16 operands everywhere (fp32 PSUM accumulate); halves DMA and enables
  2x DVE modes + FWL weight loads.
- Scores computed as S^T[k, q] = K @ Q^T with heads PAIRED: two K=64
  matmuls run concurrently in the PE array via tile_position row groups
  (rows 0:63 head-even, 64:127 head-odd).
- Causal trimming at 128-column granularity: diagonal key-tiles stream
  only the valid q range; one [128,128] step mask handles the boundary.
- Phase-1 transposes done by the DMA XBAR engine (dma_start_transpose)
  through contiguous staging tiles - zero PE/DVE cost.
- Softmax denominators ride the PV matmul as a ones-column (row 64 of
  y_ps), are gathered 16-to-a-partition, reciprocal'd in one wide DVE op,
  and broadcast back across partitions with tiny K=16 selector matmuls.
- Phase interleaving: QKV tiles for block j+1 and the output projection
  for block j-1 are emitted around attention block j so every engine
  queue stays dense (keeps the PE HAM-warm).
"""

import math

import numpy as np
import ml_dtypes

import concourse.bacc as bacc
import concourse.tile as tile
from concourse import mybir
from concourse.bass import ts
from concourse.bass_utils import run_bass_kernel_spmd

# Problem dims (hardcoded per contract).
B, S, D, H, KV, HD = 2, 2048, 1024, 16, 4, 64
NH = H // KV          # 4 query heads per core (one KV group)
GD = NH * HD          # 256 out-dims of Wq per group
NQKV = GD + 2 * HD    # 384
P = 128               # partitions
NST = S // P          # 16 sequence tiles
JW = 512              # query-block width for attention
NJ = S // JW          # 4 query blocks
NC = 8                # cores
ROPE_BASE = 10000.0
RMS_EPS = 1.1920929e-07
F32 = mybir.dt.float32
BF16 = mybir.dt.bfloat16
AXX = mybir.AxisListType.X
ACT = mybir.ActivationFunctionType
ALU = mybir.AluOpType

bfloat16 = ml_dtypes.bfloat16


def _build_program():
    nc = bacc.Bacc("TRN2", target_bir_lowering=False, debug=False)

    xT = nc.dram_tensor("xT", [D, S], BF16, kind="ExternalInput").ap()
    wqkv = nc.dram_tensor("wqkv", [D, NQKV], BF16, kind="ExternalInput").ap()
    wp2 = nc.dram_tensor("wp2", [P, 2 * D], BF16, kind="ExternalInput").ap()
    cosn = nc.dram_tensor("cosn", [P, NST * HD], BF16, kind="ExternalInput").ap()
    sinpm = nc.dram_tensor("sinpm", [P, NST * HD], BF16, kind="ExternalInput").ap()
    maskt = nc.dram_tensor("maskt", [P, P], BF16, kind="ExternalInput").ap()
    selq = nc.dram_tensor("selq", [16, 2 * NJ * P], BF16, kind="ExternalInput").ap()
    qg8 = nc.dram_tensor("qg8", [1, NH], F32, kind="ExternalInput").ap()
    ypt = nc.dram_tensor("ypt", [D, S], BF16, kind="ExternalOutput").ap()

    with tile.TileContext(nc) as tc:
        _body(tc, xT, wqkv, wp2, cosn, sinpm, maskt, selq, qg8, ypt)
    nc.compile()
    return nc


def _body(tc, xT, wqkv, wp2, cosn, sinpm, maskt, selq, qg8, ypt):
    nc = tc.nc

    with (
        tc.tile_pool(name="consts", bufs=1) as consts,
        tc.tile_pool(name="work", bufs=2) as work,
        tc.tile_pool(name="p2p", bufs=3) as p2p,
        tc.tile_pool(name="stgp", bufs=3) as stgp,
        tc.tile_pool(name="mmp", bufs=2, space="PSUM") as mmp,
        tc.tile_pool(name="stp", bufs=2, space="PSUM") as stp,
        tc.tile_pool(name="yp", bufs=1, space="PSUM") as yp,
    ):
        # ---------------- persistent SBUF state ----------------
        xT_sb = consts.tile([P, 8, S], BF16, name="xT_sb")
        wqkv_sb = consts.tile([P, 8, NQKV], BF16, name="wqkv_sb")
        wp_sb = consts.tile([P, 2, D], BF16, name="wp_sb")
        cos_sb = consts.tile([P, NST, HD], BF16, name="cos_sb")
        sin_sb = consts.tile([P, NST, HD], BF16, name="sin_sb")
        mask_sb = consts.tile([P, P], BF16, name="mask_sb")
        selq_sb = consts.tile([16, 2, NJ, P], BF16, name="selq_sb")
        qg8_sb = consts.tile([P, NH], F32, name="qg8_sb")
        # attention operand layouts (head pairs stacked on partitions)
        qT2_sb = consts.tile([P, 2, S], BF16, name="qT2_sb")
        kT2_sb = consts.tile([P, S], BF16, name="kT2_sb")
        v_sb = consts.tile([P, NST, HD + 1], BF16, name="v_sb")
        y_sb = consts.tile([P, 2, S], BF16, name="y_sb")
        den_sb = consts.tile([16, NJ, P], BF16, name="den_sb")

        xTr = xT.rearrange("(c p) s -> p c s", p=P)
        for c in range(8):
            nc.sync.dma_start(out=xT_sb[:, c, :], in_=xTr[:, c, :])
        nc.sync.dma_start(out=wqkv_sb, in_=wqkv.rearrange("(c p) n -> p c n", p=P))
        nc.sync.dma_start(out=wp_sb, in_=wp2.rearrange("p (c m) -> p c m", c=2))
        nc.sync.dma_start(out=cos_sb, in_=cosn.rearrange("p (t f) -> p t f", f=HD))
        nc.sync.dma_start(out=sin_sb, in_=sinpm.rearrange("p (t f) -> p t f", f=HD))
        nc.sync.dma_start(out=mask_sb, in_=maskt)
        nc.sync.dma_start(
            out=selq_sb, in_=selq.rearrange("r (c j p) -> r c j p", c=2, j=NJ)
        )
        nc.gpsimd.dma_start(out=qg8_sb, in_=qg8.to_broadcast([P, NH]))
        nc.vector.memset(v_sb[:, :, HD : HD + 1], 1.0)

        # ---------------- phase 1: QKV + RMS + RoPE + transpose ----------
        def phase1(i):
            qkv = mmp.tile([P, 512], F32, name=f"qkv{i}", tag="mm")
            for cc in range(8):
                nc.tensor.matmul(
                    qkv[:, 0:NQKV],
                    lhsT=xT_sb[:, cc, ts(i, P)],
                    rhs=wqkv_sb[:, cc, :],
                    start=(cc == 0),
                    stop=(cc == 7),
                )
            # v tile (bf16) straight out of PSUM
            nc.vector.tensor_copy(v_sb[:, i, 0:HD], qkv[:, 5 * HD : NQKV])

            q5 = qkv[:, 0 : 5 * HD].rearrange("p (s d) -> p s d", d=HD)
            sq = work.tile([P, 5, HD], F32, name=f"sq{i}", tag="sq")
            nc.scalar.square(sq, q5)
            ss = work.tile([P, 5], F32, name=f"ss{i}", tag="ss")
            nc.vector.reduce_sum(ss, sq, axis=AXX)
            m5 = work.tile([P, 5], F32, name=f"m5{i}", tag="m5")
            nc.vector.tensor_scalar(
                out=m5, in0=ss, scalar1=1.0 / HD, scalar2=RMS_EPS,
                op0=ALU.mult, op1=ALU.add,
            )
            s5 = work.tile([P, 5], F32, name=f"s5{i}", tag="s5")
            nc.scalar.activation(s5, m5, ACT.Sqrt)
            r5 = work.tile([P, 5], F32, name=f"r5{i}", tag="r5")
            nc.vector.reciprocal(r5, s5)
            nc.vector.tensor_mul(r5[:, 0:NH], r5[:, 0:NH], qg8_sb)

            qks = work.tile([P, 5, HD], BF16, name=f"qks{i}", tag="qks")
            nc.vector.tensor_mul(qks, q5, r5[:, :, None].broadcast_to([P, 5, HD]))
            tcos = work.tile([P, 5, HD], BF16, name=f"tcos{i}", tag="tcos")
            nc.vector.tensor_mul(
                tcos, qks, cos_sb[:, i, :][:, None, :].broadcast_to([P, 5, HD])
            )
            tsin = work.tile([P, 5, HD], BF16, name=f"tsin{i}", tag="tsin")
            qks_swap = qks.rearrange("p s (h w) -> p s h w", h=2)[:, :, ::-1, :]
            sin_b = (
                sin_sb[:, i, :][:, None, :]
                .broadcast_to([P, 5, HD])
                .rearrange("p s (h w) -> p s h w", h=2)
            )
            nc.vector.tensor_mul(
                tsin.rearrange("p s (h w) -> p s h w", h=2), qks_swap, sin_b
            )
            rot = work.tile([P, 6, HD], BF16, name=f"rot{i}", tag="rot")
            nc.vector.tensor_add(rot[:, 0:5], tcos, tsin)
            nc.vector.tensor_add(rot[:, 5:6], tcos[:, 4:5], tsin[:, 4:5])

            # DMA XBAR transposes through contiguous staging tiles
            for c in range(2):
                stg = stgp.tile([P, P], BF16, name=f"st{i}_{c}", tag="stg")
                nc.sync.dma_start_transpose(
                    out=stg, in_=rot[:, 2 * c : 2 * c + 2, :].rearrange("p a b -> p (a b)")
                )
                nc.sync.dma_start(out=qT2_sb[:, c, ts(i, P)], in_=stg)
            stgk = stgp.tile([P, P], BF16, name=f"stk{i}", tag="stg")
            nc.sync.dma_start_transpose(
                out=stgk, in_=rot[:, 4:6, :].rearrange("p a b -> p (a b)")
            )
            nc.sync.dma_start(out=kT2_sb[:, ts(i, P)], in_=stgk)

        # ---------------- phase 2: attention for (pair c, block j) -------
        mask2 = mask_sb[:, None, :].broadcast_to([P, 2, P])

        def attention(c, j):
            jq0 = j * JW
            nt = 4 * (j + 1)
            y_ps = yp.tile([HD + 1, 2, JW], F32, name=f"y{c}{j}", tag="y")
            for t in range(nt):
                m = t - 4 * j
                qlo = P * m if m >= 0 else 0
                st = stp.tile([P, 2, JW], F32, name=f"s{c}{j}{t}", tag="st")
                for h in range(2):
                    base = HD * h
                    nc.tensor.matmul(
                        st[:, h, qlo:JW],
                        lhsT=kT2_sb[base : base + HD, ts(t, P)],
                        rhs=qT2_sb[base : base + HD, c, jq0 + qlo : jq0 + JW],
                        start=True,
                        stop=True,
                        tile_position=(base, 0),
                    )
                p2 = p2p.tile([P, 2, JW], BF16, name=f"p{c}{j}{t}", tag="p2")
                nc.scalar.activation(p2[:, :, qlo:JW], st[:, :, qlo:JW], ACT.Exp)
                if m >= 0:
                    nc.vector.tensor_mul(
                        p2[:, :, qlo : qlo + P], p2[:, :, qlo : qlo + P], mask2
                    )
                for h in range(2):
                    nc.tensor.matmul(
                        y_ps[:, h, qlo:JW],
                        lhsT=v_sb[:, t, :],
                        rhs=p2[:, h, qlo:JW],
                        start=(t == 0),
                        stop=(t == nt - 1),
                    )
            # unnormalized y + denominators out of PSUM in one copy
            stg = work.tile([HD + 1, 2, JW], BF16, name=f"ys{c}{j}", tag="ystg")
            nc.vector.tensor_copy(stg, y_ps)
            nc.sync.dma_start(out=y_sb[0:HD, c, ts(j, JW)], in_=stg[0:HD, 0, :])
            nc.sync.dma_start(out=y_sb[HD:P, c, ts(j, JW)], in_=stg[0:HD, 1, :])
            for h in range(2):
                head = 2 * c + h
                nc.sync.dma_start(
                    out=den_sb[4 * head : 4 * head + 4, j, :],
                    in_=stg[HD : HD + 1, h, :],
                )

        # ---------------- phase 3: normalize + output projection ---------
        def normproj(j):
            rden = work.tile([16, P], F32, name=f"rd{j}", tag="rden")
            nc.vector.reciprocal(rden, den_sb[:, j, :])
            rdb = work.tile([16, P], BF16, name=f"rb{j}", tag="rdb")
            nc.vector.tensor_copy(rdb, rden)
            y2s = []
            for c in range(2):
                rbc = mmp.tile([P, 512], F32, name=f"rbc{c}{j}", tag="mm")
                for qq in range(NJ):
                    nc.tensor.matmul(
                        rbc[:, ts(qq, P)],
                        lhsT=selq_sb[:, c, qq, :],
                        rhs=rdb,
                        start=True,
                        stop=True,
                    )
                y2 = work.tile([P, JW], BF16, name=f"y2{c}{j}", tag=f"y2_{c}")
                nc.vector.tensor_mul(y2, y_sb[:, c, ts(j, JW)], rbc)
                y2s.append(y2)
            for mc in range(D // P):
                op = mmp.tile([P, 512], F32, name=f"op{mc}{j}", tag="mm")
                for c in range(2):
                    nc.tensor.matmul(
                        op,
                        lhsT=wp_sb[:, c, ts(mc, P)],
                        rhs=y2s[c],
                        start=(c == 0),
                        stop=(c == 1),
                    )
                ob = work.tile([P, JW], BF16, name=f"ob{mc}{j}", tag="ob")
                nc.vector.tensor_copy(ob, op)
                nc.sync.dma_start(out=ypt[ts(mc, P), ts(j, JW)], in_=ob)

        # ---------------- emission schedule ------------------------------
        for i in range(4):
            phase1(i)
        for i in range(4, 8):
            phase1(i)
        for j in range(NJ):
            for c in range(2):
                attention(c, j)
            if j < 2:
                for i in range(4 * j + 8, 4 * j + 12):
                    phase1(i)
            normproj(j)


_PROG = None


def _get_program():
    global _PROG
    if _PROG is None:
        _PROG = _build_program()
    return _PROG


def _host_tables():
    inv_freq = 1.0 / (ROPE_BASE ** (np.arange(0, HD, 2, dtype=np.float32) / HD))
    t = np.arange(S, dtype=np.float32)
    freqs = t[:, None] * inv_freq[None, :].astype(np.float32)  # [S, 32]
    cosf = np.cos(freqs).astype(np.float32)
    sinf = np.sin(freqs).astype(np.float32)
    cosd = np.concatenate([cosf, cosf], axis=1)          # [S, 64]
    sind = np.concatenate([sinf, -sinf], axis=1)         # [S, 64] sign baked
    cosn = np.ascontiguousarray(
        cosd.reshape(NST, P, HD).transpose(1, 0, 2).reshape(P, NST * HD)
    ).astype(bfloat16)
    sinpm = np.ascontiguousarray(
        sind.reshape(NST, P, HD).transpose(1, 0, 2).reshape(P, NST * HD)
    ).astype(bfloat16)
    p_idx = np.arange(P)[:, None]
    c_idx = np.arange(P)[None, :]
    maskt = (c_idx >= p_idx).astype(bfloat16)            # [128, 128]
    # selectors: selq[r, c, qq, p] = 1 iff r == 4*(2c + p//64) + qq
    selq = np.zeros((16, 2, NJ, P), dtype=bfloat16)
    for c in range(2):
        for qq in range(NJ):
            for p in range(P):
                selq[4 * (2 * c + p // HD) + qq, c, qq, p] = 1.0
    selq = np.ascontiguousarray(selq.reshape(16, 2 * NJ * P))
    return cosn, sinpm, maskt, selq


def _in_maps(x, Wq, Wk, Wv, Wproj, q_gain):
    cosn, sinpm, maskt, selq = _host_tables()
    maps = []
    for core in range(NC):
        b, g = divmod(core, KV)
        xTc = np.ascontiguousarray(x[b].T).astype(bfloat16)  # [D, S]
        wqkv = np.ascontiguousarray(
            np.concatenate(
                [
                    Wq[g * GD : (g + 1) * GD].T,
                    Wk[g * HD : (g + 1) * HD].T,
                    Wv[g * HD : (g + 1) * HD].T,
                ],
                axis=1,
            )
        ).astype(bfloat16)  # [D, 384]
        wsl = Wproj[:, g * GD : (g + 1) * GD].T.reshape(NH, HD, D)  # [head, d, m]
        wp2 = np.ascontiguousarray(
            np.stack(
                [
                    np.concatenate([wsl[0], wsl[1]], axis=0),
                    np.concatenate([wsl[2], wsl[3]], axis=0),
                ],
                axis=1,
            ).reshape(P, 2 * D)
        ).astype(bfloat16)
        qg8 = np.ascontiguousarray(
            (q_gain[g * NH : (g + 1) * NH] / 8.0).astype(np.float32).reshape(1, NH)
        )
        maps.append(
            {
                "xT": xTc,
                "wqkv": wqkv,
                "wp2": wp2,
                "cosn": cosn,
                "sinpm": sinpm,
                "maskt": maskt,
                "selq": selq,
                "qg8": qg8,
            }
        )
    return maps


def kernel(x, Wq, Wk, Wv, Wproj, q_gain, _collect=None):
    x = np.asarray(x, dtype=np.float32)
    Wq = np.asarray(Wq, dtype=np.float32)
    Wk = np.asarray(Wk, dtype=np.float32)
    Wv = np.asarray(Wv, dtype=np.float32)
    Wproj = np.asarray(Wproj, dtype=np.float32)
    q_gain = np.asarray(q_gain, dtype=np.float32)

    nc = _get_program()
    maps = _in_maps(x, Wq, Wk, Wv, Wproj, q_gain)
    res = run_bass_kernel_spmd(nc, maps, core_ids=list(range(NC)))
    if _collect is not None:
        _collect.append(res)

    out = np.zeros((B, S, D), dtype=np.float64)
    for core in range(NC):
        b, _ = divmod(core, KV)
        out[b] += res.results[core]["ypt"].T.astype(np.float64)
    return out.astype(np.float32)


# revision 4
# speedup vs baseline: 1.6546x; 1.3288x over previous
"""Trainium2 Bass kernel for a causal self-attention block (GQA + per-head
RMS-norm + RoPE + learned q-gain), sharded over 8 NeuronCores.

Sharding: data-parallel over batch (B=2) x tensor-parallel over KV groups
(4 groups of 4 query heads). core = b*4 + g. Each core computes full
attention for its 4 heads and a partial output projection (its 256 in-dims
of Wproj); the host sums the 4 partials per batch element.

v3 design:
- bf16 operands everywhere (fp32 PSUM accumulate).
- Scores as S^T[k, q] = K @ Q^T with heads PAIRED: two K=64 matmuls run
  concurrently in the PE via tile_position row groups.
- Causal trimming at 128-column granularity; one [128,128] step mask for
  the diagonal boundary.
- Phase-1 transposes: ONE fused DMA XBAR transpose per s-tile
  ([128,384] -> [128,3,128] block transpose) writing q-pair/k slabs
  directly into the merged qkT layout. Zero PE/DVE cost, 16 DMA ops.
- RMS rsqrt via Ln+Exp so the whole kernel uses a single ACT table set
  (natural_log_exp_and_others covers Exp/Ln/Square/Copy).
- Softmax denominators ride the PV matmul as a ones-column, gathered
  16-per-partition, one wide reciprocal, broadcast back via K=16 selector
  matmuls, normalization fused into the output-projection preamble.
- Phase interleaving keeps every engine queue dense.
"""

import math

import numpy as np
import ml_dtypes

import concourse.bacc as bacc
import concourse.tile as tile
from concourse import mybir
from concourse.bass import ts
from concourse.bass_utils import run_bass_kernel_spmd

# Problem dims (hardcoded per contract).
B, S, D, H, KV, HD = 2, 2048, 1024, 16, 4, 64
NH = H // KV          # 4 query heads per core (one KV group)
GD = NH * HD          # 256 out-dims of Wq per group
NQKV = GD + 2 * HD    # 384
P = 128               # partitions
NST = S // P          # 16 sequence tiles
JW = 512              # query-block width for attention
NJ = S // JW          # 4 query blocks
NC = 8                # cores
ROPE_BASE = 10000.0
RMS_EPS = 1.1920929e-07
F32 = mybir.dt.float32
BF16 = mybir.dt.bfloat16
AXX = mybir.AxisListType.X
ACT = mybir.ActivationFunctionType
ALU = mybir.AluOpType

bfloat16 = ml_dtypes.bfloat16


def _build_program():
    nc = bacc.Bacc("TRN2", target_bir_lowering=False, debug=False)

    xT = nc.dram_tensor("xT", [D, S], BF16, kind="ExternalInput").ap()
    wqkv = nc.dram_tensor("wqkv", [D, NQKV], BF16, kind="ExternalInput").ap()
    wp2 = nc.dram_tensor("wp2", [P, 2 * D], BF16, kind="ExternalInput").ap()
    cosn = nc.dram_tensor("cosn", [P, NST * HD], BF16, kind="ExternalInput").ap()
    sinpm = nc.dram_tensor("sinpm", [P, NST * HD], BF16, kind="ExternalInput").ap()
    maskt = nc.dram_tensor("maskt", [P, P], BF16, kind="ExternalInput").ap()
    selq = nc.dram_tensor("selq", [16, 2 * NJ * P], BF16, kind="ExternalInput").ap()
    qg8 = nc.dram_tensor("qg8", [1, NH], BF16, kind="ExternalInput").ap()
    ypt = nc.dram_tensor("ypt", [D, S], BF16, kind="ExternalOutput").ap()

    with tile.TileContext(nc) as tc:
        _body(tc, xT, wqkv, wp2, cosn, sinpm, maskt, selq, qg8, ypt)
    nc.compile()
    return nc


def _body(tc, xT, wqkv, wp2, cosn, sinpm, maskt, selq, qg8, ypt):
    nc = tc.nc

    with (
        tc.tile_pool(name="consts", bufs=1) as consts,
        tc.tile_pool(name="work", bufs=2) as work,
        tc.tile_pool(name="p2p", bufs=3) as p2p,
        tc.tile_pool(name="mmp", bufs=2, space="PSUM") as mmp,
        tc.tile_pool(name="stp", bufs=2, space="PSUM") as stp,
        tc.tile_pool(name="yp", bufs=1, space="PSUM") as yp,
    ):
        # ---------------- persistent SBUF state ----------------
        xT_sb = consts.tile([P, 8, S], BF16, name="xT_sb")
        wqkv_sb = consts.tile([P, 8, NQKV], BF16, name="wqkv_sb")
        wp_sb = consts.tile([P, 2, D], BF16, name="wp_sb")
        cos_sb = consts.tile([P, NST, HD], BF16, name="cos_sb")
        sin_sb = consts.tile([P, NST, HD], BF16, name="sin_sb")
        mask_sb = consts.tile([P, P], BF16, name="mask_sb")
        selq_sb = consts.tile([16, 2, NJ, P], BF16, name="selq_sb")
        qg8_sb = consts.tile([P, NH], BF16, name="qg8_sb")
        # merged attention operand layout: [:, i, 0/1, :] = qT pair c tile i,
        # [:, i, 2, :] = kT duplicated into both partition halves.
        qkT_sb = consts.tile([P, NST, 3, P], BF16, name="qkT_sb")
        v_sb = consts.tile([P, NST, HD + 1], BF16, name="v_sb")
        y_sb = consts.tile([P, 2, S], BF16, name="y_sb")
        den_sb = consts.tile([16, NJ, P], BF16, name="den_sb")

        nc.sync.dma_start(out=wqkv_sb, in_=wqkv.rearrange("(c p) n -> p c n", p=P))
        xTr = xT.rearrange("(c p) s -> p c s", p=P)
        for c in range(8):
            nc.sync.dma_start(out=xT_sb[:, c, :], in_=xTr[:, c, :])
        nc.sync.dma_start(out=wp_sb, in_=wp2.rearrange("p (c m) -> p c m", c=2))
        nc.sync.dma_start(out=cos_sb, in_=cosn.rearrange("p (t f) -> p t f", f=HD))
        nc.sync.dma_start(out=sin_sb, in_=sinpm.rearrange("p (t f) -> p t f", f=HD))
        nc.sync.dma_start(out=mask_sb, in_=maskt)
        nc.sync.dma_start(
            out=selq_sb, in_=selq.rearrange("r (c j p) -> r c j p", c=2, j=NJ)
        )
        nc.gpsimd.dma_start(out=qg8_sb, in_=qg8.to_broadcast([P, NH]))
        nc.vector.memset(v_sb[:, :, HD : HD + 1], 1.0)

        # ------------ phase 1: QKV + RMS + RoPE + transpose (tile pairs) --
        def phase1(u):
            i0 = 2 * u
            qkvs = []
            for k in range(2):
                qkv = mmp.tile([P, 512], F32, name=f"qkv{u}{k}", tag="mm")
                for cc in range(8):
                    nc.tensor.matmul(
                        qkv[:, 0:NQKV],
                        lhsT=xT_sb[:, cc, ts(i0 + k, P)],
                        rhs=wqkv_sb[:, cc, :],
                        start=(cc == 0),
                        stop=(cc == 7),
                    )
                qkvs.append(qkv)
            qc = work.tile([P, 2, 5, HD], BF16, name=f"qc{u}", tag="qc")
            for k in range(2):
                nc.vector.tensor_copy(v_sb[:, i0 + k, 0:HD], qkvs[k][:, 5 * HD : NQKV])
                nc.vector.tensor_copy(
                    qc[:, k], qkvs[k][:, 0 : 5 * HD].rearrange("p (s d) -> p s d", d=HD)
                )
            sq = work.tile([P, 2, 5, HD], BF16, name=f"sq{u}", tag="sq")
            nc.vector.tensor_mul(sq, qc, qc)
            ss = work.tile([P, 2, 5], F32, name=f"ss{u}", tag="ss")
            nc.vector.reduce_sum(ss, sq, axis=AXX)
            m10 = work.tile([P, 2, 5], F32, name=f"m10{u}", tag="m10")
            nc.vector.tensor_scalar(
                out=m10, in0=ss, scalar1=1.0 / HD, scalar2=RMS_EPS,
                op0=ALU.mult, op1=ALU.add,
            )
            lnm = work.tile([P, 2, 5], F32, name=f"lnm{u}", tag="lnm")
            nc.scalar.activation(lnm, m10, ACT.Ln)
            r10 = work.tile([P, 2, 5], BF16, name=f"r10{u}", tag="r10")
            nc.scalar.activation(r10, lnm, ACT.Exp, scale=-0.5)
            nc.vector.tensor_mul(
                r10[:, :, 0:NH], r10[:, :, 0:NH],
                qg8_sb[:, None, :].broadcast_to([P, 2, NH]),
            )
            qks = work.tile([P, 2, 5, HD], BF16, name=f"qks{u}", tag="qks")
            nc.vector.tensor_mul(
                qks, qc, r10[:, :, :, None].broadcast_to([P, 2, 5, HD])
            )
            tcos = work.tile([P, 2, 5, HD], BF16, name=f"tcos{u}", tag="tcos")
            nc.vector.tensor_mul(
                tcos, qks,
                cos_sb[:, i0 : i0 + 2, None, :].broadcast_to([P, 2, 5, HD]),
            )
            tsin = work.tile([P, 2, 5, HD], BF16, name=f"tsin{u}", tag="tsin")
            qks_swap = qks.rearrange("p u s (h w) -> p u s h w", h=2)[:, :, :, ::-1, :]
            sin_b = (
                sin_sb[:, i0 : i0 + 2, None, :]
                .broadcast_to([P, 2, 5, HD])
                .rearrange("p u s (h w) -> p u s h w", h=2)
            )
            nc.vector.tensor_mul(
                tsin.rearrange("p u s (h w) -> p u s h w", h=2), qks_swap, sin_b
            )
            for k in range(2):
                rot = work.tile([P, 6, HD], BF16, name=f"rot{u}{k}", tag=f"rot{k}")
                nc.vector.tensor_add(rot[:, 0:5], tcos[:, k], tsin[:, k])
                nc.vector.tensor_add(rot[:, 5:6], tcos[:, k, 4:5], tsin[:, k, 4:5])
                nc.sync.dma_start_transpose(
                    out=qkT_sb[:, i0 + k, :, :],
                    in_=rot.rearrange("p a b -> p (a b)"),
                )

        # ---------------- phase 2: attention for (pair c, block j) -------
        mask2 = mask_sb[:, None, :].broadcast_to([P, 2, P])

        def attention(c, j):
            nt = 4 * (j + 1)
            y_ps = yp.tile([HD + 1, 2, JW], F32, name=f"y{c}{j}", tag="y")
            for t in range(nt):
                m = t - 4 * j
                qlo = P * m if m >= 0 else 0
                st = stp.tile([P, 2, JW], F32, name=f"s{c}{j}{t}", tag="st")
                for h in range(2):
                    base = HD * h
                    nc.tensor.matmul(
                        st[:, h, qlo:JW],
                        lhsT=qkT_sb[base : base + HD, t, 2, :],
                        rhs=qkT_sb[
                            base : base + HD, 4 * j + (qlo // P) : 4 * (j + 1), c, :
                        ],
                        start=True,
                        stop=True,
                        tile_position=(base, 0),
                    )
                p2 = p2p.tile([P, 2, JW], BF16, name=f"p{c}{j}{t}", tag="p2")
                nc.scalar.activation(p2[:, :, qlo:JW], st[:, :, qlo:JW], ACT.Exp)
                if m >= 0:
                    nc.vector.tensor_mul(
                        p2[:, :, qlo : qlo + P], p2[:, :, qlo : qlo + P], mask2
                    )
                for h in range(2):
                    nc.tensor.matmul(
                        y_ps[:, h, qlo:JW],
                        lhsT=v_sb[:, t, :],
                        rhs=p2[:, h, qlo:JW],
                        start=(t == 0),
                        stop=(t == nt - 1),
                    )
            # unnormalized y + denominators out of PSUM in one copy
            stg = work.tile([HD + 1, 2, JW], BF16, name=f"ys{c}{j}", tag="ystg")
            nc.vector.tensor_copy(stg, y_ps)
            nc.sync.dma_start(out=y_sb[0:HD, c, ts(j, JW)], in_=stg[0:HD, 0, :])
            nc.sync.dma_start(out=y_sb[HD:P, c, ts(j, JW)], in_=stg[0:HD, 1, :])
            for h in range(2):
                head = 2 * c + h
                nc.sync.dma_start(
                    out=den_sb[4 * head : 4 * head + 4, j, :],
                    in_=stg[HD : HD + 1, h, :],
                )

        # ---------------- phase 3: normalize + output projection ---------
        def normproj(j):
            rden = work.tile([16, P], F32, name=f"rd{j}", tag="rden")
            nc.vector.reciprocal(rden, den_sb[:, j, :])
            rdb = work.tile([16, P], BF16, name=f"rb{j}", tag="rdb")
            nc.vector.tensor_copy(rdb, rden)
            y2s = []
            for c in range(2):
                rbc = mmp.tile([P, 512], F32, name=f"rbc{c}{j}", tag="mm")
                for qq in range(NJ):
                    nc.tensor.matmul(
                        rbc[:, ts(qq, P)],
                        lhsT=selq_sb[:, c, qq, :],
                        rhs=rdb,
                        start=True,
                        stop=True,
                    )
                y2 = work.tile([P, JW], BF16, name=f"y2{c}{j}", tag=f"y2_{c}")
                nc.vector.tensor_mul(y2, y_sb[:, c, ts(j, JW)], rbc)
                y2s.append(y2)
            for mc in range(D // P):
                op = mmp.tile([P, 512], F32, name=f"op{mc}{j}", tag="mm")
                for c in range(2):
                    nc.tensor.matmul(
                        op,
                        lhsT=wp_sb[:, c, ts(mc, P)],
                        rhs=y2s[c],
                        start=(c == 0),
                        stop=(c == 1),
                    )
                ob = work.tile([P, JW], BF16, name=f"ob{mc}{j}", tag="ob")
                nc.vector.tensor_copy(ob, op)
                nc.sync.dma_start(out=ypt[ts(mc, P), ts(j, JW)], in_=ob)

        # ---------------- emission schedule ------------------------------
        for u in range(4):
            phase1(u)
        for j in range(NJ):
            for c in range(2):
                attention(c, j)
            normproj(j)
            if j < 2:
                phase1(4 + 2 * j)
                phase1(5 + 2 * j)


_PROG = None


def _get_program():
    global _PROG
    if _PROG is None:
        _PROG = _build_program()
    return _PROG


def _host_tables():
    inv_freq = 1.0 / (ROPE_BASE ** (np.arange(0, HD, 2, dtype=np.float32) / HD))
    t = np.arange(S, dtype=np.float32)
    freqs = t[:, None] * inv_freq[None, :].astype(np.float32)  # [S, 32]
    cosf = np.cos(freqs).astype(np.float32)
    sinf = np.sin(freqs).astype(np.float32)
    cosd = np.concatenate([cosf, cosf], axis=1)          # [S, 64]
    sind = np.concatenate([sinf, -sinf], axis=1)         # [S, 64] sign baked
    cosn = np.ascontiguousarray(
        cosd.reshape(NST, P, HD).transpose(1, 0, 2).reshape(P, NST * HD)
    ).astype(bfloat16)
    sinpm = np.ascontiguousarray(
        sind.reshape(NST, P, HD).transpose(1, 0, 2).reshape(P, NST * HD)
    ).astype(bfloat16)
    p_idx = np.arange(P)[:, None]
    c_idx = np.arange(P)[None, :]
    maskt = (c_idx >= p_idx).astype(bfloat16)            # [128, 128]
    # selectors: selq[r, c, qq, p] = 1 iff r == 4*(2c + p//64) + qq
    selq = np.zeros((16, 2, NJ, P), dtype=bfloat16)
    for c in range(2):
        for qq in range(NJ):
            for p in range(P):
                selq[4 * (2 * c + p // HD) + qq, c, qq, p] = 1.0
    selq = np.ascontiguousarray(selq.reshape(16, 2 * NJ * P))
    return cosn, sinpm, maskt, selq


def _in_maps(x, Wq, Wk, Wv, Wproj, q_gain):
    cosn, sinpm, maskt, selq = _host_tables()
    maps = []
    for core in range(NC):
        b, g = divmod(core, KV)
        xTc = np.ascontiguousarray(x[b].T).astype(bfloat16)  # [D, S]
        wqkv = np.ascontiguousarray(
            np.concatenate(
                [
                    Wq[g * GD : (g + 1) * GD].T,
                    Wk[g * HD : (g + 1) * HD].T,
                    Wv[g * HD : (g + 1) * HD].T,
                ],
                axis=1,
            )
        ).astype(bfloat16)  # [D, 384]
        wsl = Wproj[:, g * GD : (g + 1) * GD].T.reshape(NH, HD, D)  # [head, d, m]
        wp2 = np.ascontiguousarray(
            np.stack(
                [
                    np.concatenate([wsl[0], wsl[1]], axis=0),
                    np.concatenate([wsl[2], wsl[3]], axis=0),
                ],
                axis=1,
            ).reshape(P, 2 * D)
        ).astype(bfloat16)
        qg8 = np.ascontiguousarray(
            (q_gain[g * NH : (g + 1) * NH] / 8.0).reshape(1, NH)
        ).astype(bfloat16)
        maps.append(
            {
                "xT": xTc,
                "wqkv": wqkv,
                "wp2": wp2,
                "cosn": cosn,
                "sinpm": sinpm,
                "maskt": maskt,
                "selq": selq,
                "qg8": qg8,
            }
        )
    return maps


def kernel(x, Wq, Wk, Wv, Wproj, q_gain, _collect=None):
    x = np.asarray(x, dtype=np.float32)
    Wq = np.asarray(Wq, dtype=np.float32)
    Wk = np.asarray(Wk, dtype=np.float32)
    Wv = np.asarray(Wv, dtype=np.float32)
    Wproj = np.asarray(Wproj, dtype=np.float32)
    q_gain = np.asarray(q_gain, dtype=np.float32)

    nc = _get_program()
    maps = _in_maps(x, Wq, Wk, Wv, Wproj, q_gain)
    res = run_bass_kernel_spmd(nc, maps, core_ids=list(range(NC)))
    if _collect is not None:
        _collect.append(res)

    out = np.zeros((B, S, D), dtype=np.float64)
    for core in range(NC):
        b, _ = divmod(core, KV)
        out[b] += res.results[core]["ypt"].T.astype(np.float64)
    return out.astype(np.float32)
